# revision 11
# baseline (speedup 1.0000x reference)
"""Trainium2 Bass kernel for a 4-layer decoder (nn_Decoder_46531675685089).

Sharding: Megatron TP-8 (2 heads/core for attention, F/8=512 FFN cols/core).
Activations flow feature-major [d, tok] through matmuls with tok order
(rank, batch, s_local); out-proj/FFN2 use the activation tile as the
stationary operand so partial sums emerge token-major, ready for a flat
ReduceScatter. Each core then BatchNorms only its own 256-token chunk
(BN stats per-s over (b, d) are fully local after RS), transposes the
small chunk back to feature-major and AllGathers.  Matmuls and exchange
buffers are bf16 (fp32 PSUM accumulation); BN/residual math is fp32.
MHA1 computes only the unmasked first 512 key positions.  Softmax
denominators come from a ones-row appended to V; normalization uses a
K=1 broadcast matmul of the reciprocal row.
"""

import numpy as np
import ml_dtypes

import concourse.bass as bass
import concourse.mybir as mybir
import concourse.tile as tile
from concourse import bacc
from concourse.bass_utils import run_bass_kernel_spmd
from concourse.masks import make_identity

F32 = mybir.dt.float32
BF16 = mybir.dt.bfloat16
NPBF16 = ml_dtypes.bfloat16

R = 8            # cores
L = 4            # layers
B = 2            # batch
S = 1024         # sequence
D = 1024         # model dim
HLOC = 2         # heads per core
DK = 64
FLOC = 512       # FFN cols per core
CH = 128         # s positions per core
NT = B * S       # 2048 tokens, order (r, b, s_local)
NTB = S          # tokens per batch
MASK = 512       # key positions >= MASK are masked in MHA1
EPS = 1e-5

AluOp = mybir.AluOpType
Act = mybir.ActivationFunctionType


# ---------------------------------------------------------------- builder --

def build_kernel(nc):
    # ---------------- DRAM I/O ----------------
    t_in = {}
    def ein(name, shape, dt):
        t_in[name] = nc.dram_tensor(name, list(shape), dt, kind="ExternalInput")
        return t_in[name]

    x_chunk = ein("x_chunk", (B, CH, D), F32)
    xT0 = ein("xT0", (R, D, B, CH), BF16)
    wq = {i: ein(f"wq{i}", (L, 8, 128, 128), BF16) for i in (1, 2)}
    wk = {i: ein(f"wk{i}", (L, 8, 128, 128), BF16) for i in (1, 2)}
    wv = {i: ein(f"wv{i}", (L, 8, 128, 128), BF16) for i in (1, 2)}
    wo = {i: ein(f"wo{i}", (L, 128, D), BF16) for i in (1, 2)}
    bqkv = {i: ein(f"bqkv{i}", (L, 3, 128), F32) for i in (1, 2)}
    w1 = ein("w1", (L, 8, 128, FLOC), BF16)
    w2 = ein("w2", (L, 4, 128, D), BF16)
    bf1 = ein("bf1", (L, 4, 128), F32)
    bias_bc = ein("bias_bc", (L, 3, D), F32)     # bo1, bo2, bf2 rows
    gbe = ein("gbe", (L, 3, 2, CH), F32)         # per-core g/be slices

    out_chunk = nc.dram_tensor("out_chunk", [B, CH, D], F32, kind="ExternalOutput")

    # ---------------- internal DRAM (exchange) ----------------
    rs_in, rs_out, ag_in, ag_out = [], [], [], []
    for p in range(2):
        rs_in.append(nc.dram_tensor(f"rs_in{p}", [NT, D], BF16))
        rs_out.append(nc.dram_tensor(f"rs_out{p}", [B * CH, D], BF16))
        ag_in.append(nc.dram_tensor(f"ag_in{p}", [D, B, CH], BF16))
        ag_out.append(nc.dram_tensor(f"ag_out{p}", [R, D, B, CH], BF16,
                                     addr_space="Shared"))

    groups = [list(range(R))]

    with tile.TileContext(nc) as tc:
        import contextlib
        ctx = contextlib.ExitStack()
        with ctx:
            consts = ctx.enter_context(tc.tile_pool(name="consts", bufs=1))
            wpool = ctx.enter_context(tc.tile_pool(name="weights", bufs=1))
            xpool = ctx.enter_context(tc.tile_pool(name="xT", bufs=1))
            qkv_pool = ctx.enter_context(tc.tile_pool(name="qkv", bufs=1))
            attn_pool = ctx.enter_context(tc.tile_pool(name="attn", bufs=2))
            res_pool = ctx.enter_context(tc.tile_pool(name="res", bufs=3))
            chunk_pool = ctx.enter_context(tc.tile_pool(name="chunk", bufs=1))
            stat_pool = ctx.enter_context(tc.tile_pool(name="stats", bufs=4))
            rd_pool = ctx.enter_context(tc.tile_pool(name="rd", bufs=2))
            out_pool = ctx.enter_context(tc.tile_pool(name="outp", bufs=3))
            ps_mm = ctx.enter_context(tc.tile_pool(name="ps_mm", bufs=6, space="PSUM"))
            ps_sm = ctx.enter_context(tc.tile_pool(name="ps_sm", bufs=2, space="PSUM"))

            # constants
            ident32 = consts.tile([128, 128], F32)
            make_identity(nc, ident32)
            ident16 = consts.tile([128, 128], BF16)
            make_identity(nc, ident16)
            ones32 = consts.tile([1, 64], F32)
            nc.vector.memset(ones32, 1.0)
            eps_t = consts.tile([128, 1], F32)
            nc.vector.memset(eps_t, EPS)

            # ---------------- helpers ----------------
            def load_xT(src4):
                """src4: DRAM AP [R, D, B, CH] -> SBUF [128, 8, NT] feature-major."""
                xT = xpool.tile([128, 8, NT], BF16, tag="xT")
                s4 = src4.rearrange("r (jd dp) b s -> dp jd r b s", dp=128)
                for jd in range(8):
                    nc.sync.dma_start(
                        out=xT[:, jd].rearrange("p (r b s) -> p r b s", b=B, s=CH),
                        in_=s4[:, jd])
                return xT

            def proj_qkv(xT, w_sb, b_sb, ntok, tag):
                """q/k/v projection: out [128(2h*dk), ntok] bf16 (+ bias).
                n-blocked so each LDWEIGHTS serves all token blocks."""
                o_sb = qkv_pool.tile([128, NT], BF16, tag=tag, name=tag)
                nb = ntok // 512
                pss = [ps_mm.tile([128, 512], F32, tag="mm", name=f"qkvps{n}")
                       for n in range(nb)]
                for jd in range(8):
                    for n in range(nb):
                        nc.tensor.matmul(pss[n], w_sb[:, jd],
                                         xT[:, jd, 512 * n:512 * n + 512],
                                         start=(jd == 0), stop=(jd == 7))
                for n in range(nb):
                    nc.vector.tensor_scalar_add(o_sb[:, 512 * n:512 * n + 512],
                                                pss[n], b_sb)
                return o_sb

            def stage(li, si, xT, res_sb, masked=None, mha=None):
                """One Megatron stage.  Returns (new res chunk fp32, new xT or None)."""
                par = (li * 3 + si) % 2
                last = (li == L - 1 and si == 2)

                if si in (0, 1):
                    i = si + 1
                    TM = MASK * B if (si == 0) else NT    # unmasked key tokens
                    KVR = TM // (B * CH)                  # kv r-tiles per batch
                    # weights
                    wq_sb = wpool.tile([128, 8, 128], BF16, tag="wq")
                    wk_sb = wpool.tile([128, 8, 128], BF16, tag="wk")
                    wv_sb = wpool.tile([128, 8, 128], BF16, tag="wv")
                    wo_sb = wpool.tile([64, HLOC, D], BF16, tag="wo")
                    bq_sb = stat_pool.tile([128, 3], F32, tag="bqkv")
                    nc.sync.dma_start(out=wq_sb, in_=wq[i].ap()[li].rearrange("jd dp k -> dp jd k"))
                    nc.sync.dma_start(out=wk_sb, in_=wk[i].ap()[li].rearrange("jd dp k -> dp jd k"))
                    nc.sync.dma_start(out=wv_sb, in_=wv[i].ap()[li].rearrange("jd dp k -> dp jd k"))
                    nc.sync.dma_start(out=wo_sb,
                                      in_=wo[i].ap()[li].rearrange("(h p) d -> p h d", p=64))
                    nc.sync.dma_start(out=bq_sb, in_=bqkv[i].ap()[li].rearrange("n p -> p n"))

                    qT = proj_qkv(xT, wq_sb, bq_sb[:, 0:1], NT, "qT")
                    kT = proj_qkv(xT, wk_sb, bq_sb[:, 1:2], TM, "kT")
                    vT = proj_qkv(xT, wv_sb, bq_sb[:, 2:3], TM, "vT")

                    # per-head attention outputs [64, NT] bf16 (partition base 0)
                    aT_h = [qkv_pool.tile([64, NT], BF16, tag=f"aT{h}",
                                          name=f"aT{h}")
                            for h in range(HLOC)]
                    q4 = qT.rearrange("p (r b s) -> p r b s", b=B, s=CH)
                    k4 = kT.rearrange("p (r b s) -> p r b s", b=B, s=CH)
                    v4 = vT.rearrange("p (r b s) -> p r b s", b=B, s=CH)

                    for b in range(B):
                        # v tok-major: per head 65 cols [v(64) | ones] -> den row
                        vaug = attn_pool.tile([128, KVR, 130], BF16, tag="vaug")
                        nc.vector.memset(vaug, 1.0)
                        for kt in range(KVR):
                            tp = ps_sm.tile([128, 128], BF16, tag="tp")
                            nc.tensor.transpose(tp, v4[:, kt, b], ident16)
                            nc.vector.tensor_copy(vaug[:, kt, 0:64], tp[:, 0:64])
                            nc.vector.tensor_copy(vaug[:, kt, 65:129], tp[:, 64:128])
                        for h in range(HLOC):
                            hp = slice(64 * h, 64 * h + 64)
                            expT = attn_pool.tile([128, KVR, NTB], BF16, tag="expT")
                            for kt in range(KVR):
                                for n in range(2):   # q-token halves (4 r's each)
                                    sc = ps_mm.tile([128, 512], F32, tag="mm")
                                    nc.tensor.matmul(
                                        sc, k4[hp, kt, b], q4[hp, 4 * n:4 * n + 4, b],
                                        start=True, stop=True)
                                    nc.scalar.activation(
                                        expT[:, kt, 512 * n:512 * n + 512], sc,
                                        Act.Exp, scale=1.0 / np.sqrt(DK))
                            avs = [ps_mm.tile([65, 512], F32, tag="mm",
                                               name=f"avps{n}") for n in range(2)]
                            for kt in range(KVR):
                                for n in range(2):
                                    nc.tensor.matmul(
                                        avs[n], vaug[:, kt, 65 * h:65 * h + 65],
                                        expT[:, kt, 512 * n:512 * n + 512],
                                        start=(kt == 0), stop=(kt == KVR - 1))
                            for n in range(2):
                                av = avs[n]
                                avsb = attn_pool.tile([65, 512], F32, tag="avsb")
                                nc.scalar.copy(avsb, av)
                                rec = rd_pool.tile([1, 512], F32, tag="rec")
                                den = rd_pool.tile([1, 512], F32, tag="den")
                                nc.sync.dma_start(out=den, in_=avsb[64:65, :])
                                nc.vector.reciprocal_approx_fast(rec, den)
                                bc = ps_sm.tile([64, 512], F32, tag="tp")
                                nc.tensor.matmul(bc, ones32, rec, start=True, stop=True)
                                bcs = attn_pool.tile([64, 512], F32, tag="bcs")
                                nc.vector.tensor_copy(bcs, bc)
                                ah4 = aT_h[h].rearrange("p (r bb s) -> p r bb s",
                                                        bb=B, s=CH)
                                nc.vector.tensor_mul(
                                    ah4[:, 4 * n:4 * n + 4, b], avsb[0:64, :], bcs)

                    # out-proj: partial sums token-major -> rs_in
                    for m in range(16):
                        po = out_pool.tile([128, D], BF16, tag="po")
                        pss = [ps_mm.tile([128, 512], F32, tag="mm",
                                          name=f"opps{nh}") for nh in range(2)]
                        for h in range(HLOC):
                            for nh in range(2):
                                nc.tensor.matmul(
                                    pss[nh], aT_h[h][:, 128 * m:128 * m + 128],
                                    wo_sb[:, h, 512 * nh:512 * nh + 512],
                                    start=(h == 0), stop=(h == HLOC - 1))
                        nc.vector.tensor_copy(po[:, 0:512], pss[0])
                        nc.scalar.copy(po[:, 512:1024], pss[1])
                        nc.sync.dma_start(out=rs_in[par].ap()[128 * m:128 * m + 128],
                                          in_=po)
                else:
                    # FFN
                    w1_sb = wpool.tile([128, 8, FLOC], BF16, tag="w1")
                    w2_sb = wpool.tile([128, 4, D], BF16, tag="w2")
                    bf1_sb = stat_pool.tile([128, 4], F32, tag="bf1")
                    nc.sync.dma_start(out=w1_sb, in_=w1.ap()[li].rearrange("jd dp f -> dp jd f"))
                    nc.sync.dma_start(out=w2_sb, in_=w2.ap()[li].rearrange("jf fp d -> fp jf d"))
                    nc.sync.dma_start(out=bf1_sb, in_=bf1.ap()[li].rearrange("jf fp -> fp jf"))

                    hidT = attn_pool.tile([128, 4, NT], BF16, tag="expT", name="hidT")
                    for jf in range(4):
                        pss = [ps_mm.tile([128, 512], F32, tag="mm",
                                          name=f"f1ps{n}") for n in range(4)]
                        for jd in range(8):
                            for n in range(4):
                                nc.tensor.matmul(
                                    pss[n], w1_sb[:, jd, 128 * jf:128 * jf + 128],
                                    xT[:, jd, 512 * n:512 * n + 512],
                                    start=(jd == 0), stop=(jd == 7))
                        for n in range(4):
                            nc.scalar.activation(hidT[:, jf, 512 * n:512 * n + 512],
                                                 pss[n], Act.Relu,
                                                 bias=bf1_sb[:, jf:jf + 1])
                    for m in range(16):
                        po = out_pool.tile([128, D], BF16, tag="po")
                        pss = [ps_mm.tile([128, 512], F32, tag="mm",
                                          name=f"f2ps{nh}") for nh in range(2)]
                        for jf in range(4):
                            for nh in range(2):
                                nc.tensor.matmul(
                                    pss[nh], hidT[:, jf, 128 * m:128 * m + 128],
                                    w2_sb[:, jf, 512 * nh:512 * nh + 512],
                                    start=(jf == 0), stop=(jf == 3))
                        nc.vector.tensor_copy(po[:, 0:512], pss[0])
                        nc.scalar.copy(po[:, 512:1024], pss[1])
                        nc.sync.dma_start(out=rs_in[par].ap()[128 * m:128 * m + 128],
                                          in_=po)

                # ---- ReduceScatter ----
                nc.gpsimd.collective_compute(
                    "ReduceScatter", AluOp.add, replica_groups=groups,
                    ins=[rs_in[par].ap()], outs=[rs_out[par].ap()])

                # ---- chunk: bias + residual + BN ----
                ch = chunk_pool.tile([128, B, D], BF16, tag="ch")
                nc.sync.dma_start(out=ch,
                                  in_=rs_out[par].ap().rearrange("(b s) d -> s b d", b=B))
                bb = chunk_pool.tile([128, D], F32, tag="bb")
                brow = bias_bc.ap()[li, si]      # [D]
                nc.sync.dma_start(
                    out=bb,
                    in_=bass.AP(tensor=brow.tensor, offset=brow.offset,
                                ap=[[0, 128]] + brow.ap))
                g_sb = stat_pool.tile([128, 2], F32, tag="gbe")
                nc.sync.dma_start(out=g_sb, in_=gbe.ap()[li, si].rearrange("n s -> s n"))

                u = chunk_pool.tile([128, B, D], F32, tag="u")
                stats = stat_pool.tile([128, 2 * B, 6], F32, tag="bnst")
                for b in range(B):
                    nc.vector.tensor_add(u[:, b], ch[:, b], bb)
                    nc.vector.tensor_add(u[:, b], u[:, b], res_sb[:, b])
                    for half in range(2):
                        nc.vector.bn_stats(stats[:, 2 * b + half],
                                           u[:, b, 512 * half:512 * half + 512])
                mv = stat_pool.tile([128, 2], F32, tag="mv")
                nc.vector.bn_aggr(mv, stats)
                std = stat_pool.tile([128, 1], F32, tag="std")
                nc.scalar.activation(std, mv[:, 1:2], Act.Sqrt, bias=eps_t)
                rstd = stat_pool.tile([128, 1], F32, tag="rstd")
                nc.vector.reciprocal(rstd, std)
                A_t = stat_pool.tile([128, 1], F32, tag="A")
                nc.vector.tensor_mul(A_t, rstd, g_sb[:, 0:1])
                mA = stat_pool.tile([128, 1], F32, tag="mA")
                nc.vector.tensor_mul(mA, mv[:, 0:1], A_t)
                B_t = stat_pool.tile([128, 1], F32, tag="B")
                nc.vector.tensor_sub(B_t, g_sb[:, 1:2], mA)

                keep = si != 1   # y-chunk (si==1) is not a later residual
                if keep:
                    xn = res_pool.tile([128, B, D], F32, tag="res", name="xn_res")
                else:
                    xn = chunk_pool.tile([128, B, D], F32, tag="xn_tmp", name="xn_tmp")
                for b in range(B):
                    nc.vector.tensor_scalar(xn[:, b], u[:, b], A_t, B_t,
                                            AluOp.mult, AluOp.add)

                if last:
                    nc.sync.dma_start(out=out_chunk.ap().rearrange("b s d -> s b d"),
                                      in_=xn)
                    return xn, None

                # ---- transpose own chunk to feature-major, AllGather ----
                for jd in range(8):
                    tx = out_pool.tile([128, B, 128], BF16, tag="tx")
                    for b in range(B):
                        tp = ps_sm.tile([128, 128], F32, tag="tp")
                        nc.tensor.transpose(tp, xn[:, b, 128 * jd:128 * jd + 128],
                                            ident32)
                        nc.vector.tensor_copy(tx[:, b], tp)
                    nc.sync.dma_start(out=ag_in[par].ap()[128 * jd:128 * jd + 128],
                                      in_=tx)
                nc.gpsimd.collective_compute(
                    "AllGather", AluOp.bypass, replica_groups=groups,
                    ins=[ag_in[par].ap()], outs=[ag_out[par].ap()])
                xT_new = load_xT(ag_out[par].ap())
                return xn, xT_new

            # ---------------- main program ----------------
            res = res_pool.tile([128, B, D], F32, tag="res")
            nc.sync.dma_start(out=res, in_=x_chunk.ap().rearrange("b s d -> s b d"))
            xT = load_xT(xT0.ap())

            for li in range(L):
                h_res = res
                x1, xT = stage(li, 0, xT, h_res)
                _y, xT = stage(li, 1, xT, h_res)
                res, xT = stage(li, 2, xT, x1)

    return nc


# ---------------------------------------------------------------- host ----

_CACHE = {}


def _get_compiled():
    if "nc" not in _CACHE:
        nc = bacc.Bacc("TRN2", target_bir_lowering=False, debug=False,
                       num_devices=R)
        build_kernel(nc)
        nc.compile()
        _CACHE["nc"] = nc
    return _CACHE["nc"]


def _prep_core_inputs(inp, c):
    """Per-core input map (numpy)."""
    f32 = np.float32

    def bf(a):
        return np.ascontiguousarray(np.asarray(a, f32).astype(NPBF16))

    x = np.asarray(inp["x"], f32)
    m = {}
    m["x_chunk"] = np.ascontiguousarray(x[:, c * CH:(c + 1) * CH, :])
    m["xT0"] = bf(x.reshape(B, R, CH, D).transpose(1, 3, 0, 2))  # r d b s
    for i in (1, 2):
        for nm, w in (("wq", inp[f"Wq{i}"]), ("wk", inp[f"Wk{i}"]),
                      ("wv", inp[f"Wv{i}"])):
            wc = np.asarray(w, f32)[:, 2 * c:2 * c + 2]          # L 2 D 64
            wc = wc.transpose(0, 2, 1, 3).reshape(L, D, 128)     # L d (h k)
            m[f"{nm}{i}"] = bf(wc.reshape(L, 8, 128, 128))
        m[f"wo{i}"] = bf(np.asarray(inp[f"Wo{i}"], f32)[:, 128 * c:128 * c + 128, :])
        bq = np.asarray(inp[f"bq{i}"], f32)[:, 2 * c:2 * c + 2].reshape(L, 128)
        bk = np.asarray(inp[f"bk{i}"], f32)[:, 2 * c:2 * c + 2].reshape(L, 128)
        bv = np.asarray(inp[f"bv{i}"], f32)[:, 2 * c:2 * c + 2].reshape(L, 128)
        m[f"bqkv{i}"] = np.ascontiguousarray(np.stack([bq, bk, bv], axis=1))
    m["w1"] = bf(np.asarray(inp["W1"], f32)[:, :, FLOC * c:FLOC * (c + 1)]
                 .reshape(L, 8, 128, FLOC))
    m["w2"] = bf(np.asarray(inp["W2"], f32)[:, FLOC * c:FLOC * (c + 1), :]
                 .reshape(L, 4, 128, D))
    m["bf1"] = np.ascontiguousarray(
        np.asarray(inp["bf1"], f32)[:, FLOC * c:FLOC * (c + 1)].reshape(L, 4, 128))
    m["bias_bc"] = np.ascontiguousarray(np.stack(
        [np.asarray(inp["bo1"], f32), np.asarray(inp["bo2"], f32),
         np.asarray(inp["bf2"], f32)], axis=1))
    sl = slice(CH * c, CH * (c + 1))
    m["gbe"] = np.ascontiguousarray(np.stack(
        [np.stack([np.asarray(inp[f"g{j}"], f32)[:, sl],
                   np.asarray(inp[f"be{j}"], f32)[:, sl]], axis=1)
         for j in (1, 2, 3)], axis=1))
    return m


def kernel(**inputs):
    nc = _get_compiled()
    in_maps = [_prep_core_inputs(inputs, c) for c in range(R)]
    res = run_bass_kernel_spmd(nc, in_maps, list(range(R)))
    chunks = [res.results[c]["out_chunk"] for c in range(R)]
    out = np.concatenate(chunks, axis=1).astype(np.float32)
    return out
